# revision 35
# baseline (speedup 1.0000x reference)
"""3-layer GCN (DGL GraphConv norm='both') on 8 TRN2 NeuronCores.

Strategy (edge-cut, dst-owner sharding):
  - Permute the 100k nodes into 960 degree-balanced slices of 128 dst slots
    (120 slices per core, piece-major table layout).
  - Node features live replicated per core in DRAM ("xn" tables, bf16,
    pre-scaled by the src-side degree norm). The table is viewed as 4 chunks
    of 30720 rows so gather indices fit in int16 for dma_gather (<= 32767).
  - Per (slice, chunk) the incident edges are padded to 512 gather slots
    (pad: idx 0 / dstrel -1); a gather call covers a slice PAIR (1024 idx).
    The gathered layout puts edge j in partition j%128, block j//128 --
    exactly matmul lhsT tiles. Rows are 256B bf16 (dma_gather minimum).
  - Segment-sum via one-hot S_T matmuls accumulated in PSUM:
        acc[fi, slot] += gathered_tile.T @ S_T
    S_T is HOST-BAKED (graph structure is layer-invariant) as an fp8e4
    input tensor (0.0/1.0 exact), streamed one pair (32 tiles, 512KB) at a
    time via HWDGE, and fed to the PE directly as a mixed-dtype matmul
    operand (bf16 lhsT x fp8 rhs). No on-device one-hot generation (was the
    #1 cost: 3.7ms of DVE is_equal) and no cast pass.
  - Per slice: out[slot, fo] = acc.T @ W via a second matmul, then fused
    relu(out * norm) on the scalar engine producing the bf16 row for the
    next layer's table. Layers 1-2 fold the next layer's src-norm into the
    same scale; layer 3 adds the bias and emits fp32.
  - An 8-core AllGather (bf16, Shared output => mesh direct-write algo)
    rebuilds the replicated xn table between layers. (GNN_PIECES>1 splits
    it into piece AGs that fire mid-layer and gate only the next layer's
    matching gather chunks; with the gather-locality fixes, 4 pieces
    measured fastest.)
"""

import os as _os

import numpy as np
import ml_dtypes

import concourse.bass as bass
import concourse.mybir as mybir
import concourse.tile as tile
from concourse import bacc, library_config
from concourse.bass_utils import run_bass_kernel_spmd

BF16 = ml_dtypes.bfloat16
FP8 = ml_dtypes.float8_e4m3

P = 128
NCORES = 8
N_NODES = 100000
SLICES_TOTAL = 960
SLICES_CORE = SLICES_TOTAL // NCORES  # 120
N_PAD = SLICES_TOTAL * P  # 122880
PER_CORE = SLICES_CORE * P  # 15360
NCHUNK = 4
CHUNK_ROWS = N_PAD // NCHUNK  # 30720 < 32768
TSC = 4  # gather tiles per (slice, chunk)
SLOT_CAP = TSC * P  # 512 edge slots per (slice, chunk)
NPAIR = SLICES_CORE // 2  # 60 slice pairs
NI_CALL = 2 * SLOT_CAP  # 1024 = dma_gather per-call limit
F_IN = 128
F_HID = 128
F_OUT = 64

TILES_PAIR = NCHUNK * 2 * TSC  # 32 gather tiles per pair
ST_COLS = NPAIR * TILES_PAIR * P  # 245760 one-hot columns
IDX_COLS = SLICES_CORE * NCHUNK * SLOT_CAP // 16  # 15360 idx columns

# AllGather pipelining: the table is laid out piece-major, so the AG for
# piece p can fire as soon as every core finished its local slices of that
# piece and the next layer's gathers for the covered chunks gate on it.
# NPIECE=1 degenerates to a single end-of-layer AllGather.
NPIECE = int(_os.environ.get("GNN_PIECES", "4"))
# S_T tiles for the first N_ST_CACHE pairs stay resident in SBUF (loaded
# once at startup) - removes their HBM re-reads in every layer and lets
# the first post-AllGather pairs start with zero S_T latency.
N_ST_CACHE = int(_os.environ.get("GNN_ST_CACHE", "24"))
SL_PIECE = SLICES_CORE // NPIECE  # local slices per piece per core
PIECE_CORE_ROWS = SL_PIECE * P
PAIRS_PIECE = NPAIR // NPIECE
CPP = NCHUNK // NPIECE  # gather chunks per piece
PIECE_ROWS = N_PAD // NPIECE


def _gslice(core, r):
    """Local slice r of `core` -> global table slice (piece-major layout)."""
    return (r // SL_PIECE) * (NCORES * SL_PIECE) + core * SL_PIECE + (r % SL_PIECE)

_DBG_LAYERS = int(_os.environ.get("GNN_LAYERS", "3"))
_DBG_PAIRS = int(_os.environ.get("GNN_PAIRS", str(NPAIR)))
_DBG_NO_AG = int(_os.environ.get("GNN_NO_AG", "0"))


def _preprocess(src, dst):
    """Permutation + per-bucket padded gather indices / dst slots."""
    n = N_NODES
    deg_out = np.bincount(src, minlength=n).astype(np.float32)
    deg_in = np.bincount(dst, minlength=n).astype(np.float32)
    ns = 1.0 / np.sqrt(np.maximum(deg_out, 1.0))
    nd = 1.0 / np.sqrt(np.maximum(deg_in, 1.0))

    # Degree-balanced slice assignment: snake round-robin over slices in
    # descending in-degree order. perm[node] = slice*128 + slot.
    order = np.argsort(-deg_in, kind="stable")
    slice_of = np.empty(n, dtype=np.int64)
    slot_of = np.empty(n, dtype=np.int64)
    for k in range(0, n, SLICES_TOTAL):
        stratum = order[k : k + SLICES_TOTAL]
        slot = k // SLICES_TOTAL
        m = len(stratum)
        if (slot % 2) == 0:
            slices = np.arange(m)
        else:
            slices = SLICES_TOTAL - 1 - np.arange(m)
        slice_of[stratum] = slices
        slot_of[stratum] = slot
    # Snake index s -> (core k, local slice r) -> piece-major table slice g.
    k_of = slice_of % NCORES
    r_of = slice_of // NCORES
    g_of = (
        (r_of // SL_PIECE) * (NCORES * SL_PIECE) + k_of * SL_PIECE + (r_of % SL_PIECE)
    )
    perm = g_of * P + slot_of  # table row (gather layout)
    perm_out = (k_of * SLICES_CORE + r_of) * P + slot_of  # output row

    s_perm = perm[src]
    d_perm = perm[dst]
    chunk = s_perm // CHUNK_ROWS
    idxval = (s_perm % CHUNK_ROWS).astype(np.int16)
    slot = (d_perm % P).astype(np.int16)
    key = (d_perm // P) * NCHUNK + chunk  # (global slice, chunk) bucket

    nbuckets = SLICES_TOTAL * NCHUNK
    counts = np.bincount(key, minlength=nbuckets)
    if counts.max() > SLOT_CAP:
        raise RuntimeError(f"bucket overflow: {counts.max()} > {SLOT_CAP}")
    # Sort by (bucket, src row): each gather lane then sweeps ascending
    # addresses, improving DRAM page locality of the random 256B reads.
    eorder = np.lexsort((idxval, key))
    offs = np.zeros(nbuckets + 1, dtype=np.int64)
    np.cumsum(counts, out=offs[1:])
    pos = np.arange(len(src)) - offs[key[eorder]]

    idx_pad = np.zeros((nbuckets, SLOT_CAP), dtype=np.int16)
    rel_pad = np.full((nbuckets, SLOT_CAP), -1, dtype=np.int16)
    idx_pad[key[eorder], pos] = idxval[eorder]
    rel_pad[key[eorder], pos] = slot[eorder]
    # Pad slots replicate the bucket's own real indices instead of all
    # hammering row 0 of the chunk: spreads pad reads across DRAM banks
    # like real traffic (their one-hot columns stay all-zero via rel=-1).
    cnt = np.maximum(counts, 1)
    j = np.arange(SLOT_CAP)
    wrap = np.mod(j[None, :], cnt[:, None])  # [nbuckets, SLOT_CAP]
    rows = np.arange(nbuckets)[:, None]
    padmask = j[None, :] >= cnt[:, None]
    src_idx = idx_pad[rows, wrap]
    idx_pad = np.where(padmask & (counts[:, None] > 0), src_idx, idx_pad)

    return perm, perm_out, ns, nd, idx_pad, rel_pad


def _wrap16(flat):
    """[NI] int16 -> [128, NI//16]: element j at [j%16, j//16], replicated x8."""
    w = flat.reshape(-1, 16).T
    return np.tile(w, (8, 1))


def _core_arrays(core, perm_out, ns, nd, idx_pad, rel_pad):
    """Per-core idx / one-hot S_T / norm arrays matching the device loop."""
    idx_all = np.empty((P, IDX_COLS), dtype=np.int16)
    st_all = np.zeros((P, ST_COLS), dtype=FP8)
    icol = 0
    slot_iota = np.arange(P, dtype=np.int16)
    for pr in range(NPAIR):
        for c in range(NCHUNK):
            flat = np.concatenate(
                [idx_pad[_gslice(core, 2 * pr + si) * NCHUNK + c] for si in range(2)]
            )
            idx_all[:, icol : icol + NI_CALL // 16] = _wrap16(flat)
            icol += NI_CALL // 16
        # One-hot blocks, pair-major layout: (pr, c, si, t) -> [e, slot]
        base = pr * TILES_PAIR * P
        for c in range(NCHUNK):
            for si in range(2):
                rel = rel_pad[_gslice(core, 2 * pr + si) * NCHUNK + c]  # [512]
                for t in range(TSC):
                    tb = base + ((c * 2 + si) * TSC + t) * P
                    blk = rel[t * P : (t + 1) * P]  # [128] edge slots
                    st_all[:, tb : tb + P] = (
                        blk[:, None] == slot_iota[None, :]
                    ).astype(FP8)
    assert icol == IDX_COLS

    nds12 = np.zeros((P, SLICES_CORE), dtype=np.float32)
    nd3 = np.zeros((P, SLICES_CORE), dtype=np.float32)
    base = core * PER_CORE
    mask = (perm_out >= base) & (perm_out < base + PER_CORE)
    local = perm_out[mask] - base
    nds12[local % P, local // P] = (nd * ns)[mask]
    nd3[local % P, local // P] = nd[mask]
    return idx_all, st_all, nds12, nd3


def _build_program():
    nc = bacc.Bacc(
        "TRN2",
        target_bir_lowering=False,
        debug=False,
        num_devices=NCORES,
        num_swdge_queues=4,
    )
    dt = mybir.dt

    xn0_in = nc.declare_dram_parameter("xn0", [N_PAD, F_IN], dt.bfloat16, isOutput=False)
    idx_in = nc.declare_dram_parameter("idx", [P, IDX_COLS], dt.int16, isOutput=False)
    st_in = nc.declare_dram_parameter("st", [P, ST_COLS], dt.float8e4, isOutput=False)
    w1_in = nc.declare_dram_parameter("w1", [F_IN, F_HID], dt.bfloat16, isOutput=False)
    w2_in = nc.declare_dram_parameter("w2", [F_HID, F_HID], dt.bfloat16, isOutput=False)
    w3_in = nc.declare_dram_parameter("w3", [F_HID, F_OUT], dt.bfloat16, isOutput=False)
    b3_in = nc.declare_dram_parameter("b3rep", [P, F_OUT], dt.float32, isOutput=False)
    nds12_in = nc.declare_dram_parameter(
        "nds12", [P, SLICES_CORE], dt.float32, isOutput=False
    )
    nd3_in = nc.declare_dram_parameter(
        "nd3", [P, SLICES_CORE], dt.float32, isOutput=False
    )
    out_ext = nc.declare_dram_parameter("out", [PER_CORE, F_OUT], dt.float32, isOutput=True)

    with tile.TileContext(nc) as tc:
        with (
            tc.tile_pool(name="consts", bufs=1) as consts,
            tc.tile_pool(name="gt", bufs=4) as gtp,
            tc.tile_pool(name="stp", bufs=3) as stp,
            tc.tile_pool(name="work", bufs=8) as work,
            tc.tile_pool(name="outw", bufs=3) as outw,
            tc.tile_pool(name="psum", bufs=6, space="PSUM") as psum,
            tc.tile_pool(name="psw", bufs=2, space="PSUM") as psw,
            tc.tile_pool(name="dram", bufs=1, space="DRAM") as dram,
        ):
            nc.gpsimd.load_library(library_config.mlp)

            idx_t = consts.tile([P, IDX_COLS], dt.int16)
            w1_t = consts.tile([F_IN, F_HID], dt.bfloat16)
            w2_t = consts.tile([F_HID, F_HID], dt.bfloat16)
            w3_t = consts.tile([F_HID, F_OUT], dt.bfloat16)
            b3_t = consts.tile([P, F_OUT], dt.float32)
            nds12_t = consts.tile([P, SLICES_CORE], dt.float32)
            nd3_t = consts.tile([P, SLICES_CORE], dt.float32)
            stc_t = consts.tile([P, N_ST_CACHE * TILES_PAIR * P], dt.float8e4)
            nc.sync.dma_start(out=idx_t[:], in_=idx_in[:])
            nc.sync.dma_start(out=w1_t[:], in_=w1_in[:])
            nc.sync.dma_start(out=w2_t[:], in_=w2_in[:])
            nc.sync.dma_start(out=w3_t[:], in_=w3_in[:])
            nc.sync.dma_start(out=b3_t[:], in_=b3_in[:])
            nc.sync.dma_start(out=nds12_t[:], in_=nds12_in[:])
            nc.sync.dma_start(out=nd3_t[:], in_=nd3_in[:])
            nc.sync.dma_start(
                out=stc_t[:], in_=st_in[:, : N_ST_CACHE * TILES_PAIR * P]
            )

            ag_in1 = [
                dram.tile([PIECE_CORE_ROWS, F_HID], dt.bfloat16, tag=f"ag_in1_{p}", name=f"ag_in1_{p}")
                for p in range(NPIECE)
            ]
            ag_in2 = [
                dram.tile([PIECE_CORE_ROWS, F_HID], dt.bfloat16, tag=f"ag_in2_{p}", name=f"ag_in2_{p}")
                for p in range(NPIECE)
            ]
            xn1 = [
                dram.tile(
                    [PIECE_ROWS, F_HID],
                    dt.bfloat16,
                    tag=f"xn1_{p}",
                    name=f"xn1_{p}",
                    addr_space="Shared",
                )
                for p in range(NPIECE)
            ]
            xn2 = [
                dram.tile(
                    [PIECE_ROWS, F_HID],
                    dt.bfloat16,
                    tag=f"xn2_{p}",
                    name=f"xn2_{p}",
                    addr_space="Shared",
                )
                for p in range(NPIECE)
            ]
            xn0 = [
                xn0_in[c * CHUNK_ROWS : (c + 1) * CHUNK_ROWS, :] for c in range(NCHUNK)
            ]
            xn1a = [
                xn1[c // CPP][(c % CPP) * CHUNK_ROWS : (c % CPP + 1) * CHUNK_ROWS, :]
                for c in range(NCHUNK)
            ]
            xn2a = [
                xn2[c // CPP][(c % CPP) * CHUNK_ROWS : (c % CPP + 1) * CHUNK_ROWS, :]
                for c in range(NCHUNK)
            ]

            layers = [
                (xn0, w1_t, F_HID, nds12_t, ag_in1, xn1),
                (xn1a, w2_t, F_HID, nds12_t, ag_in2, xn2),
                (xn2a, w3_t, F_OUT, nd3_t, None, None),
            ]

            tc._gnn = (gtp, stp, work, outw, psum, psw, idx_t, st_in, stc_t, b3_t, out_ext)
            _emit_layers(nc, tc, layers)
    nc.compile()
    return nc


def _emit_layers(nc, tc, layers):
    dt = mybir.dt
    gtp, stp, work, outw, psum, psw, idx_t, st_in, stc_t, b3_t, out_ext = tc._gnn
    for li, (table, w_t, fo, scale_t, ag_in, ag_out) in enumerate(layers):
        if li >= _DBG_LAYERS:
            break
        with nc.named_scope(f"layer{li + 1}"):
            icol = 0
            for pr in range(NPAIR):
                if pr >= _DBG_PAIRS:
                    break
                if pr < N_ST_CACHE:
                    st_t = stc_t[
                        :, pr * TILES_PAIR * P : (pr + 1) * TILES_PAIR * P
                    ]
                else:
                    st_t = stp.tile(
                        [P, TILES_PAIR * P], dt.float8e4, tag="st_t"
                    )
                    nc.sync.dma_start(
                        out=st_t[:],
                        in_=st_in[
                            :, pr * TILES_PAIR * P : (pr + 1) * TILES_PAIR * P
                        ],
                    )
                gts = []
                for c in range(NCHUNK):
                    gt = gtp.tile([P, 2 * TSC, P], dt.bfloat16, tag=f"gt{c}")
                    nc.gpsimd.dma_gather(
                        gt[:],
                        table[c],
                        idx_t[:, icol : icol + NI_CALL // 16],
                        NI_CALL,
                        NI_CALL,
                        P,
                        queue_num=c,
                    )
                    icol += NI_CALL // 16
                    gts.append(gt)
                st3 = st_t.rearrange("p (b s) -> p b s", s=P)
                for si in range(2):
                    s = 2 * pr + si
                    acc = psum.tile([P, P], dt.float32, space="PSUM", tag="acc")
                    for c in range(NCHUNK):
                        for t in range(TSC):
                            nc.tensor.matmul(
                                out=acc[:],
                                lhsT=gts[c][:, si * TSC + t, :],
                                rhs=st3[:, (c * 2 + si) * TSC + t, :],
                                start=(c == 0 and t == 0),
                                stop=(c == NCHUNK - 1 and t == TSC - 1),
                            )
                    aggT = work.tile([P, P], dt.bfloat16, tag="aggT")
                    nc.vector.tensor_copy(out=aggT[:], in_=acc[:])
                    op = psw.tile([P, fo], dt.float32, space="PSUM", tag="op")
                    nc.tensor.matmul(
                        out=op[:], lhsT=aggT[:], rhs=w_t[:], start=True, stop=True
                    )
                    if li < 2:
                        o = outw.tile([P, fo], dt.bfloat16, tag="o")
                        nc.scalar.activation(
                            out=o[:],
                            in_=op[:],
                            func=mybir.ActivationFunctionType.Relu,
                            scale=scale_t[:, s : s + 1],
                        )
                        roff = (s % SL_PIECE) * P
                        nc.sync.dma_start(
                            out=ag_in[s // SL_PIECE][roff : roff + P, :], in_=o[:]
                        )
                    else:
                        o = outw.tile([P, fo], dt.float32, tag="o")
                        nc.scalar.activation(
                            out=o[:],
                            in_=op[:],
                            func=mybir.ActivationFunctionType.Copy,
                            scale=scale_t[:, s : s + 1],
                        )
                        nc.vector.tensor_add(out=o[:], in0=o[:], in1=b3_t[:])
                        nc.sync.dma_start(
                            out=out_ext[s * P : (s + 1) * P, :], in_=o[:]
                        )
                if (
                    ag_in is not None
                    and not _DBG_NO_AG
                    and (pr + 1) % PAIRS_PIECE == 0
                ):
                    p = pr // PAIRS_PIECE
                    nc.gpsimd.collective_compute(
                        "AllGather",
                        mybir.AluOpType.bypass,
                        replica_groups=[list(range(NCORES))],
                        ins=[ag_in[p].opt()],
                        outs=[ag_out[p].opt()],
                    )


def _make_in_maps(x, src, dst, W1, W2, W3, b3):
    perm, perm_out, ns, nd, idx_pad, rel_pad = _preprocess(src, dst)

    xn0 = np.zeros((N_PAD, F_IN), dtype=BF16)
    xn0[perm] = (x * ns[:, None]).astype(BF16)
    b3rep = np.broadcast_to(b3, (P, F_OUT)).copy()

    in_maps = []
    for c in range(NCORES):
        idx_all, st_all, nds12, nd3 = _core_arrays(
            c, perm_out, ns, nd, idx_pad, rel_pad
        )
        in_maps.append(
            {
                "xn0": xn0,
                "idx": idx_all,
                "st": st_all,
                "w1": W1.astype(BF16),
                "w2": W2.astype(BF16),
                "w3": W3.astype(BF16),
                "b3rep": b3rep,
                "nds12": nds12,
                "nd3": nd3,
            }
        )
    return in_maps, perm_out


def kernel(x, src, dst, W1, W2, W3, b3):
    x = np.ascontiguousarray(np.asarray(x, dtype=np.float32))
    src = np.asarray(src).astype(np.int64)
    dst = np.asarray(dst).astype(np.int64)
    W1 = np.ascontiguousarray(np.asarray(W1, dtype=np.float32))
    W2 = np.ascontiguousarray(np.asarray(W2, dtype=np.float32))
    W3 = np.ascontiguousarray(np.asarray(W3, dtype=np.float32))
    b3 = np.ascontiguousarray(np.asarray(b3, dtype=np.float32))

    in_maps, perm = _make_in_maps(x, src, dst, W1, W2, W3, b3)
    nc = _build_program()
    res = run_bass_kernel_spmd(nc, in_maps, list(range(NCORES)))
    global LAST_RES
    LAST_RES = res

    full = np.concatenate([res.results[c]["out"] for c in range(NCORES)], axis=0)
    return full[perm].astype(np.float32)


LAST_RES = None
